# revision 21
# baseline (speedup 1.0000x reference)
"""BlipAttention kernel for 8 Trainium2 NeuronCores (v2).

Data-parallel over batch (16 batches -> 2 per core), no collectives.

v2 strategy (vs v1): keep the PE dense end-to-end so the HAM clock gate
never re-throttles (v1 ran ~half the kernel at 1.2 GHz), and make the
scalar engine do nothing but softmax exp.

  - x is transposed to feature-major x^T ON THE HOST and uploaded bf16
    (stage-A PE transposes and the fp32 x upload are gone).
  - 4-phase software pipeline, interleaved at EMISSION level (the Tile
    scheduler's per-engine ready-heaps pop in emission order):
      P1: v-proj (both batches) + q|k-proj(b0), coarsely interleaved
      P2: attention(b0) with q|k-proj(b1) matmuls as PE filler between
          the softmax dependency stalls
      P3: attention(b1) with out-proj(b0) as PE filler
      P4: out-proj(b1)
  - scores/PV/C1 psum tiles span TWO PSUM banks ([128,1024] f32) so each
    (head, k-tile) needs ONE exp activation over the full 578-token span
    (ACT per-instruction overhead is 352 cycles -- halving the count
    saves ~45us), and chunk matmuls share each LDWEIGHTS load.
  - every PSUM->SBUF drain is on the vector engine (tensor_scalar_add /
    tensor_copy), leaving ACT 100% for exp.
  - weights are host-prepacked into contiguous per-stripe layouts so all
    weight DMAs are large and contiguous.
  - softmax denominators come free from ones-columns in the v tiles
    (PV emits sum_k exp at psum partition 96), reciprocal on DVE,
    broadcast across partitions with a rank-1 (K=1) matmul.
"""

import contextlib
from collections import deque

import numpy as np
import ml_dtypes

import concourse.bass as bass
import concourse.tile as tile
from concourse import bacc, mybir
from concourse.bass_utils import run_bass_kernel_spmd

F32 = mybir.dt.float32
F32R = mybir.dt.float32r
BF16 = mybir.dt.bfloat16
AF = mybir.ActivationFunctionType

N_CORES = 8
B_TOTAL, S, D = 16, 577, 1408
H, HD = 16, 88
SCALE = HD ** -0.5
B = B_TOTAL // N_CORES          # batches per core = 2
T = B * S                       # tokens per core = 1154
SP = S + 1                      # padded per-batch token span = 578
KT = D // 128                   # 11 k-tiles over D
MT = 2 * KT                     # 22 m-tiles over the packed q|k features
TT = (S + 127) // 128           # 5 token tiles per batch
VG = 97                         # v group width per head: 88 v cols + 9 ones
DEN = 96                        # psum partition of the softmax denominator

TOK = [(tt, tt * 128, min(128, S - tt * 128)) for tt in range(TT)]
CH_D = [(0, 512), (512, 512), (1024, 384)]    # chunks over 1408 v-features
DCG = [(0, 1024), (1024, 384)]                # out-proj column groups


class Filler:
    """Queue of emission generators; take(n) emits ~n PE-cycles of filler."""

    def __init__(self):
        self.q = deque()
        self.credit = 0

    def add(self, gen):
        # prime: first yield emits the piece's DMA prefetches only
        try:
            next(gen)
            self.q.append(gen)
        except StopIteration:
            pass

    def take(self, n):
        self.credit += n
        while self.credit > 0 and self.q:
            try:
                self.credit -= next(self.q[0])
            except StopIteration:
                self.q.popleft()

    def drain(self):
        while self.q:
            try:
                next(self.q[0])
            except StopIteration:
                self.q.popleft()


def build_program():
    nc = bacc.Bacc("TRN2", target_bir_lowering=False, debug=False,
                   num_devices=N_CORES)

    xT_ap = nc.dram_tensor("xT_bf", [D, B * SP], BF16, kind="ExternalInput").ap()
    wqkm_ap = nc.dram_tensor("wqk_m", [128, MT * KT * 128], BF16,
                             kind="ExternalInput").ap()
    wv_ap = nc.dram_tensor("wv_r", [128, KT * D], BF16, kind="ExternalInput").ap()
    wp_ap = nc.dram_tensor("wp_r", [128, KT * D], BF16, kind="ExternalInput").ap()
    bqk_ap = nc.dram_tensor("b_qk_col", [128, MT], F32, kind="ExternalInput").ap()
    bv_ap = nc.dram_tensor("b_v_row", [1, D], BF16, kind="ExternalInput").ap()
    bp_ap = nc.dram_tensor("b_p_row", [1, D], BF16, kind="ExternalInput").ap()
    ones_bf_ap = nc.dram_tensor("ones_bf", [1, 128], BF16, kind="ExternalInput").ap()
    out_ap = nc.dram_tensor("out", [T, D], F32, kind="ExternalOutput").ap()

    with tile.TileContext(nc) as tc, contextlib.ExitStack() as ctx:
        p_xT = ctx.enter_context(tc.tile_pool(name="xT", bufs=2 * KT))
        p_w = ctx.enter_context(tc.tile_pool(name="w", bufs=KT))
        p_wqk = ctx.enter_context(tc.tile_pool(name="wqk", bufs=3))
        p_vsb = ctx.enter_context(tc.tile_pool(name="vsb", bufs=2 * TT))
        p_qk = ctx.enter_context(tc.tile_pool(name="qk", bufs=2 * H))
        p_qksb = ctx.enter_context(tc.tile_pool(name="qksb", bufs=2))
        p_expT = ctx.enter_context(tc.tile_pool(name="expT", bufs=6))
        p_pvs = ctx.enter_context(tc.tile_pool(name="pvs", bufs=2))
        p_at = ctx.enter_context(tc.tile_pool(name="at", bufs=2))
        p_apk = ctx.enter_context(tc.tile_pool(name="apk", bufs=2 * KT))
        p_ost = ctx.enter_context(tc.tile_pool(name="ost", bufs=2))
        p_rec = ctx.enter_context(tc.tile_pool(name="rec", bufs=2))
        p_bias = ctx.enter_context(tc.tile_pool(name="bias", bufs=1))
        p_brow = ctx.enter_context(tc.tile_pool(name="brow", bufs=2))
        p_const = ctx.enter_context(tc.tile_pool(name="const", bufs=1))

        # PSUM: 8 banks = 2x2 (sc: scores 2-deep + pb) + 1x2 (pv) + 1x2 (big)
        p_sc = ctx.enter_context(tc.tile_pool(name="psc", bufs=2, space="PSUM"))
        p_pv = ctx.enter_context(tc.tile_pool(name="ppv", bufs=1, space="PSUM"))
        p_big = ctx.enter_context(tc.tile_pool(name="pbig", bufs=1, space="PSUM"))

        # ---- x^T tiles for b0 FIRST: stage B's critical path ----
        xTb = {0: [None] * KT, 1: [None] * KT}
        for k in range(KT):
            t = p_xT.tile([128, SP], BF16, tag="xT", name=f"xT_0_{k}")
            eng = nc.sync if k % 2 == 0 else nc.scalar
            eng.dma_start(t[:], xT_ap[k * 128:(k + 1) * 128, 0:SP])
            xTb[0][k] = t

        # ---- constants (not needed until ~12us in) ----
        ones_bf = p_const.tile([1, 128], BF16, tag="ones_bf")
        nc.sync.dma_start(ones_bf[:], ones_bf_ap[0:1, :])
        bvr = p_brow.tile([1, D], BF16, tag="brow", name="bvr")
        nc.sync.dma_start(bvr[:], bv_ap[:])
        bpr = p_brow.tile([1, D], BF16, tag="brow", name="bpr")
        nc.sync.dma_start(bpr[:], bp_ap[:])
        bq_all = p_bias.tile([128, MT], F32, tag="bias", name="bq_all")
        nc.sync.dma_start(bq_all[:], bqk_ap[:])

        # ---- x^T tiles for b1 ----
        for k in range(KT):
            t = p_xT.tile([128, SP], BF16, tag="xT", name=f"xT_1_{k}")
            eng = nc.sync if k % 2 == 0 else nc.scalar
            eng.dma_start(t[:], xT_ap[k * 128:(k + 1) * 128, SP:2 * SP])
            xTb[1][k] = t

        # ---- resident v-proj weights (11 tiles; wp reuses the slots) ----
        wv = []
        for k in range(KT):
            t = p_w.tile([128, D], BF16, tag="w", name=f"wv{k}")
            nc.gpsimd.dma_start(t[:], wv_ap[:, k * D:(k + 1) * D])
            wv.append(t)

        # ---- v tiles: ones-filled via gpsimd memset (denominator cols) ----
        vsb = {}
        for b in range(B):
            vsb[b] = []
            for tt in range(TT):
                t = p_vsb.tile([128, H * VG], BF16, tag="vsb",
                               name=f"vsb_{b}_{tt}")
                nc.vector.memset(t[:], 1.0)
                vsb[b].append(t)

        qkt = {}

        def qk_tile(b, which, h):
            key = (b, which, h)
            if key not in qkt:
                qkt[key] = p_qk.tile([HD, SP], BF16, tag="qk",
                                     name=f"qk_{b}_{which}_{h}")
            return qkt[key]

        apk = {b: [p_apk.tile([128, SP], BF16, tag="apk", name=f"apk_{b}_{k}")
                   for k in range(KT)] for b in range(B)}

        # ================= piece generators =================

        def gen_b_chunk(b, ci):
            """v projection for batch b, feature chunk ci (token-major)."""
            c0, w = CH_D[ci]
            boff = b * SP
            yield 0
            sA = p_sc.tile([128, 1024], F32, tag="ps", name=f"bps_{b}_{ci}_a")
            sB = p_sc.tile([128, 1024], F32, tag="ps", name=f"bps_{b}_{ci}_b")
            sC = p_pv.tile([128, 1024], F32, tag="ps", name=f"bps_{b}_{ci}_c")
            slot = [(sA, 0), (sA, 512), (sB, 0), (sB, 512), (sC, 0)]
            for k in range(KT):
                for tt, t0, ts in TOK:
                    tl, off = slot[tt]
                    nc.tensor.matmul(tl[0:ts, off:off + w],
                                     xTb[b][k][:, t0:t0 + ts],
                                     wv[k][:, c0:c0 + w],
                                     start=(k == 0), stop=False)
                yield TT * w
            for tt, t0, ts in TOK:
                tl, off = slot[tt]
                nc.tensor.matmul(tl[0:ts, off:off + w], ones_bf[:, 0:ts],
                                 bvr[:, c0:c0 + w], start=False, stop=True)
            yield TT * w
            h0, h1 = c0 // HD, min(H - 1, (c0 + w - 1) // HD)
            for tt, t0, ts in TOK:
                tl, off = slot[tt]
                for h in range(h0, h1 + 1):
                    s0, s1 = max(c0, h * HD), min(c0 + w, (h + 1) * HD)
                    if s1 <= s0:
                        continue
                    nc.vector.tensor_copy(
                        vsb[b][tt][0:ts, h * VG + (s0 - h * HD):
                                   h * VG + (s1 - h * HD)],
                        tl[0:ts, off + (s0 - c0):off + (s1 - c0)])
                yield 0

        def gen_c1_m(b, m):
            """q|k projection m-tile (feature-major) + head redistribution."""
            wq = p_wqk.tile([128, KT * 128], BF16, tag="wqk", name=f"wq_{b}_{m}")
            nc.scalar.dma_start(wq[:], wqkm_ap[:, m * 1408:(m + 1) * 1408])
            yield 0
            boff = b * SP
            pt = p_big.tile([128, 1024], F32, tag="ps", name=f"c1p_{b}_{m}")
            for k in range(KT):
                nc.tensor.matmul(pt[0:128, 0:512], wq[:, k * 128:(k + 1) * 128],
                                 xTb[b][k][:, 0:512],
                                 start=(k == 0), stop=(k == KT - 1))
                nc.tensor.matmul(pt[0:128, 512:578], wq[:, k * 128:(k + 1) * 128],
                                 xTb[b][k][:, 512:578],
                                 start=(k == 0), stop=(k == KT - 1))
                yield SP
            qksb = p_qksb.tile([128, SP], BF16, tag="qksb", name=f"qs_{b}_{m}")
            nc.vector.tensor_scalar_add(qksb[:, 0:SP], pt[0:128, 0:SP],
                                        bq_all[:, m:m + 1])
            which = 0 if m < KT else 1
            f_lo = (m - which * KT) * 128
            f_hi = f_lo + 128
            for h in range(f_lo // HD, min(H, (f_hi + HD - 1) // HD)):
                s0, s1 = max(f_lo, h * HD), min(f_hi, (h + 1) * HD)
                if s1 <= s0:
                    continue
                t = qk_tile(b, which, h)
                r0 = s0 - h * HD
                nc.sync.dma_start(t[r0:r0 + (s1 - s0), :],
                                  qksb[s0 - f_lo:s1 - f_lo, :])
            yield 0

        wp = []

        def emit_wp_load():
            k = len(wp)
            if k < KT:
                t = p_w.tile([128, D], BF16, tag="w", name=f"wp{k}")
                nc.gpsimd.dma_start(t[:], wp_ap[:, k * D:(k + 1) * D])
                wp.append(t)

        def gen_d_piece(b, tt, t0, ts, cgi, pool):
            """out-proj for batch b, token tile tt, column group cgi."""
            c0, w = DCG[cgi]
            yield 0
            po = pool.tile([128, 1024], F32, tag="ps", name=f"po_{b}_{tt}_{cgi}")
            for k in range(KT):
                if w == 1024:
                    nc.tensor.matmul(po[0:ts, 0:512], apk[b][k][:, t0:t0 + ts],
                                     wp[k][:, c0:c0 + 512],
                                     start=(k == 0), stop=False)
                    nc.tensor.matmul(po[0:ts, 512:1024], apk[b][k][:, t0:t0 + ts],
                                     wp[k][:, c0 + 512:c0 + 1024],
                                     start=(k == 0), stop=False)
                else:
                    nc.tensor.matmul(po[0:ts, 0:w], apk[b][k][:, t0:t0 + ts],
                                     wp[k][:, c0:c0 + w],
                                     start=(k == 0), stop=False)
                yield w
            if w == 1024:
                nc.tensor.matmul(po[0:ts, 0:512], ones_bf[:, 0:ts],
                                 bpr[:, c0:c0 + 512], start=False, stop=True)
                nc.tensor.matmul(po[0:ts, 512:1024], ones_bf[:, 0:ts],
                                 bpr[:, c0 + 512:c0 + 1024], start=False, stop=True)
            else:
                nc.tensor.matmul(po[0:ts, 0:w], ones_bf[:, 0:ts],
                                 bpr[:, c0:c0 + w], start=False, stop=True)
            ot = p_ost.tile([128, 1024], F32, tag="ost", name=f"ot_{b}_{tt}_{cgi}")
            nc.vector.tensor_copy(ot[0:ts, 0:w], po[0:ts, 0:w])
            nc.sync.dma_start(out_ap[b * S + t0:b * S + t0 + ts, c0:c0 + w],
                              ot[0:ts, 0:w])
            yield 0

        # ================= attention =================

        def finish_head(b, h, rec, pvs):
            pb = p_sc.tile([128, 1024], F32, tag="ps", name=f"pb_{b}_{h}")
            nc.tensor.matmul(pb[0:HD, 0:512], ones_bf[:, 0:HD],
                             rec[:, 0:512], start=True, stop=True)
            nc.tensor.matmul(pb[0:HD, 512:578], ones_bf[:, 0:HD],
                             rec[:, 512:578], start=True, stop=True)
            at = p_at.tile([HD, SP], BF16, tag="at", name=f"at_{b}_{h}")
            nc.vector.tensor_mul(at[:, 0:SP], pvs[0:HD, 0:SP], pb[0:HD, 0:SP])
            f0 = h * HD
            k0, r0 = f0 // 128, f0 % 128
            n0 = min(HD, 128 - r0)
            nc.sync.dma_start(apk[b][k0][r0:r0 + n0, :], at[0:n0, :])
            if n0 < HD:
                nc.sync.dma_start(apk[b][k0 + 1][0:HD - n0, :], at[n0:HD, :])

        def attention_phase(b, filler, fill_exp, fill_pv, gate=None):
            pend = None
            for h in range(H):
                if gate is not None:
                    gate(h)
                qh_t = qk_tile(b, 0, h)
                kh_t = qk_tile(b, 1, h)
                expTs = []
                for tt, t0, ts in TOK:
                    pt = p_sc.tile([128, 1024], F32, tag="ps",
                                   name=f"sc_{b}_{h}_{tt}")
                    nc.tensor.matmul(pt[0:ts, 0:512], kh_t[:, t0:t0 + ts],
                                     qh_t[:, 0:512], start=True, stop=True)
                    nc.tensor.matmul(pt[0:ts, 512:578], kh_t[:, t0:t0 + ts],
                                     qh_t[:, 512:578], start=True, stop=True)
                    et = p_expT.tile([128, SP], BF16, tag="expT",
                                     name=f"et_{b}_{h}_{tt}")
                    nc.scalar.activation(et[0:ts, 0:SP], pt[0:ts, 0:SP],
                                         AF.Exp, scale=SCALE)
                    expTs.append(et)
                    filler.take(fill_exp)
                pv = p_pv.tile([128, 1024], F32, tag="ps", name=f"pv_{b}_{h}")
                for tt, t0, ts in TOK:
                    et = expTs[tt]
                    vsl = vsb[b][tt][0:ts, h * VG:(h + 1) * VG]
                    nc.tensor.matmul(pv[0:VG, 0:512], vsl, et[0:ts, 0:512],
                                     start=(tt == 0), stop=(tt == TT - 1))
                    nc.tensor.matmul(pv[0:VG, 512:578], vsl, et[0:ts, 512:578],
                                     start=(tt == 0), stop=(tt == TT - 1))
                    filler.take(fill_pv)
                # one copy brings values AND the denominator row to SBUF
                pvs = p_pvs.tile([VG, SP], F32, tag="pvs", name=f"pvs_{b}_{h}")
                nc.vector.tensor_copy(pvs[:, 0:SP], pv[0:VG, 0:SP])
                # custom-DVE op needs base partition 0: run it over all VG
                # rows (same per-lane cost); only row 96 (the denominator)
                # is meaningful.
                recf = p_rec.tile([VG, SP], F32, tag="recf", name=f"recf_{b}_{h}")
                nc.vector.reciprocal_approx_fast(recf[:, 0:SP], pvs[:, 0:SP])
                rec = p_rec.tile([1, SP], BF16, tag="rec", name=f"rec_{b}_{h}")
                nc.vector.tensor_copy(rec[:, 0:SP], recf[DEN:DEN + 1, 0:SP])
                if pend is not None:
                    finish_head(b, *pend)
                pend = (h, rec, pvs)
            finish_head(b, *pend)

        # ================= phase drivers =================

        def drain(gen):
            for _ in gen:
                pass

        # ---- P1: v-proj(b0,b1) zipped with q|k-proj(b0) at ~1.2us grain ----
        # Fine interleave keeps the PE fed while the single big-psum slot
        # drains between C1 m-tiles (and vice versa for B's psum slots).
        fB = Filler()
        for b in range(B):
            for ci in range(3):
                fB.add(gen_b_chunk(b, ci))
        fC = Filler()
        for m in range(MT):
            fC.add(gen_c1_m(0, m))
        fB.take(31000)          # all of B(b0, chunk0) before any C1
        while fB.q or fC.q:
            fB.take(3100)
            fC.take(2560)

        # ---- P2: attention(b0) with q|k-proj(b1) as filler ----
        f2 = Filler()
        c1b1 = [gen_c1_m(1, m) for m in range(MT)]
        gstate = {"j": 0}

        for _ in range(KT):
            emit_wp_load()

        def gate2(h):
            jm = min((88 * (h + 2)) // 128, KT - 1)
            while gstate["j"] <= jm:
                j = gstate["j"]
                f2.add(c1b1[j])          # q m-tile j
                f2.add(c1b1[KT + j])     # k m-tile j
                gstate["j"] += 1

        attention_phase(0, f2, fill_exp=1100, fill_pv=650, gate=gate2)
        f2.drain()

        # ---- P3: attention(b1) with out-proj(b0) as filler ----
        f3 = Filler()
        for tt, t0, ts in TOK:
            for cgi in range(2):
                f3.add(gen_d_piece(0, tt, t0, ts, cgi, p_big))
        attention_phase(1, f3, fill_exp=650, fill_pv=390)
        f3.drain()

        # ---- P4: out-proj(b1), psum rotating across all three pools ----
        pools4 = [p_big, p_sc, p_pv]
        pi = 0
        d4 = []
        for tt, t0, ts in TOK:
            for cgi in range(2):
                d4.append(gen_d_piece(1, tt, t0, ts, cgi, pools4[pi % 3]))
                pi += 1
        primed = 0
        for i in range(len(d4)):
            while primed < min(i + 2, len(d4)):
                try:
                    next(d4[primed])
                except StopIteration:
                    pass
                primed += 1
            drain(d4[i])

    nc.compile()
    return nc


_NC_CACHE = None


def _get_nc():
    global _NC_CACHE
    if _NC_CACHE is None:
        _NC_CACHE = build_program()
    return _NC_CACHE


def make_in_maps(hidden_states, w_qkv, b_qkv, w_proj, b_proj):
    bf16 = ml_dtypes.bfloat16
    hs = np.asarray(hidden_states, dtype=np.float32)
    w_qkv = np.ascontiguousarray(np.asarray(w_qkv, dtype=np.float32))
    b_qkv = np.asarray(b_qkv, dtype=np.float32)
    w_proj = np.ascontiguousarray(np.asarray(w_proj, dtype=np.float32))
    b_proj = np.asarray(b_proj, dtype=np.float32)

    # q|k weights -> m-stripe layout: wqk_m[p, m*1408 + k*128 + c]
    #   = w_qkv[k*128 + p, m*128 + c]
    wqk = w_qkv[:, :2 * D].reshape(KT, 128, MT, 128)
    wqk_m = np.ascontiguousarray(
        wqk.transpose(1, 2, 0, 3).reshape(128, MT * KT * 128)).astype(bf16)
    # v / proj weights -> k-stripe layout: w_r[p, k*1408 + c] = w[k*128+p, c]
    wv_r = np.ascontiguousarray(
        w_qkv[:, 2 * D:].reshape(KT, 128, D).transpose(1, 0, 2)
        .reshape(128, KT * D)).astype(bf16)
    wp_r = np.ascontiguousarray(
        w_proj.reshape(KT, 128, D).transpose(1, 0, 2)
        .reshape(128, KT * D)).astype(bf16)

    bqk_m = np.ascontiguousarray(
        b_qkv[:2 * D].reshape(MT, 128).T)                 # [128, MT]
    bv_row = np.ascontiguousarray(b_qkv[2 * D:].reshape(1, D)).astype(bf16)
    bp_row = np.ascontiguousarray(b_proj.reshape(1, D)).astype(bf16)
    ones_bf = np.ones((1, 128), bf16)

    in_maps = []
    for c in range(N_CORES):
        xt = np.ones((D, B * SP), bf16)
        for b in range(B):
            xs = hs[c * B + b]                       # [S, D]
            xt[:, b * SP:b * SP + S] = xs.T.astype(bf16)
        in_maps.append({
            "xT_bf": xt,
            "wqk_m": wqk_m,
            "wv_r": wv_r,
            "wp_r": wp_r,
            "b_qk_col": bqk_m,
            "b_v_row": bv_row,
            "b_p_row": bp_row,
            "ones_bf": ones_bf,
        })
    return in_maps


def kernel(hidden_states, w_qkv, b_qkv, w_proj, b_proj):
    nc = _get_nc()
    in_maps = make_in_maps(hidden_states, w_qkv, b_qkv, w_proj, b_proj)
    res = run_bass_kernel_spmd(nc, in_maps, list(range(N_CORES)))
    out = np.concatenate(
        [res.results[c]["out"].reshape(B, S, D) for c in range(N_CORES)], axis=0)
    return out.astype(np.float32)


if __name__ == "__main__":
    rng = np.random.default_rng(0)
    hs = rng.standard_normal((B_TOTAL, S, D), dtype=np.float32)
    wq = rng.standard_normal((D, 3 * D), dtype=np.float32) * D ** -0.5
    bq = rng.standard_normal(3 * D).astype(np.float32) * 0.02
    wp = rng.standard_normal((D, D), dtype=np.float32) * D ** -0.5
    bp = rng.standard_normal(D).astype(np.float32) * 0.02
    o = kernel(hidden_states=hs, w_qkv=wq, b_qkv=bq, w_proj=wp, b_proj=bp)
    print(o.shape, o.dtype)


# revision 22
# speedup vs baseline: 1.0022x; 1.0022x over previous
"""BlipAttention kernel for 8 Trainium2 NeuronCores (v2).

Data-parallel over batch (16 batches -> 2 per core), no collectives.

v2 strategy (vs v1): keep the PE dense end-to-end so the HAM clock gate
never re-throttles (v1 ran ~half the kernel at 1.2 GHz), and make the
scalar engine do nothing but softmax exp.

  - x is transposed to feature-major x^T ON THE HOST and uploaded bf16
    (stage-A PE transposes and the fp32 x upload are gone).
  - 4-phase software pipeline, interleaved at EMISSION level (the Tile
    scheduler's per-engine ready-heaps pop in emission order):
      P1: v-proj (both batches) + q|k-proj(b0), coarsely interleaved
      P2: attention(b0) with q|k-proj(b1) matmuls as PE filler between
          the softmax dependency stalls
      P3: attention(b1) with out-proj(b0) as PE filler
      P4: out-proj(b1)
  - scores/PV/C1 psum tiles span TWO PSUM banks ([128,1024] f32) so each
    (head, k-tile) needs ONE exp activation over the full 578-token span
    (ACT per-instruction overhead is 352 cycles -- halving the count
    saves ~45us), and chunk matmuls share each LDWEIGHTS load.
  - every PSUM->SBUF drain is on the vector engine (tensor_scalar_add /
    tensor_copy), leaving ACT 100% for exp.
  - weights are host-prepacked into contiguous per-stripe layouts so all
    weight DMAs are large and contiguous.
  - softmax denominators come free from ones-columns in the v tiles
    (PV emits sum_k exp at psum partition 96), reciprocal on DVE,
    broadcast across partitions with a rank-1 (K=1) matmul.
"""

import contextlib
from collections import deque

import numpy as np
import ml_dtypes

import concourse.bass as bass
import concourse.tile as tile
from concourse import bacc, mybir
from concourse.bass_utils import run_bass_kernel_spmd

F32 = mybir.dt.float32
F32R = mybir.dt.float32r
BF16 = mybir.dt.bfloat16
AF = mybir.ActivationFunctionType

N_CORES = 8
B_TOTAL, S, D = 16, 577, 1408
H, HD = 16, 88
SCALE = HD ** -0.5
B = B_TOTAL // N_CORES          # batches per core = 2
T = B * S                       # tokens per core = 1154
SP = S + 1                      # padded per-batch token span = 578
KT = D // 128                   # 11 k-tiles over D
MT = 2 * KT                     # 22 m-tiles over the packed q|k features
TT = (S + 127) // 128           # 5 token tiles per batch
VG = 97                         # v group width per head: 88 v cols + 9 ones
DEN = 96                        # psum partition of the softmax denominator

TOK = [(tt, tt * 128, min(128, S - tt * 128)) for tt in range(TT)]
CH_D = [(0, 512), (512, 512), (1024, 384)]    # chunks over 1408 v-features
DCG = [(0, 1024), (1024, 384)]                # out-proj column groups


class Filler:
    """Queue of emission generators; take(n) emits ~n PE-cycles of filler."""

    def __init__(self):
        self.q = deque()
        self.credit = 0

    def add(self, gen):
        # prime: first yield emits the piece's DMA prefetches only
        try:
            next(gen)
            self.q.append(gen)
        except StopIteration:
            pass

    def take(self, n):
        self.credit += n
        while self.credit > 0 and self.q:
            try:
                self.credit -= next(self.q[0])
            except StopIteration:
                self.q.popleft()

    def drain(self):
        while self.q:
            try:
                next(self.q[0])
            except StopIteration:
                self.q.popleft()


def build_program():
    nc = bacc.Bacc("TRN2", target_bir_lowering=False, debug=False,
                   num_devices=N_CORES)

    xT_ap = nc.dram_tensor("xT_bf", [D, B * SP], BF16, kind="ExternalInput").ap()
    wqkm_ap = nc.dram_tensor("wqk_m", [128, MT * KT * 128], BF16,
                             kind="ExternalInput").ap()
    wv_ap = nc.dram_tensor("wv_r", [128, KT * D], BF16, kind="ExternalInput").ap()
    wp_ap = nc.dram_tensor("wp_r", [128, KT * D], BF16, kind="ExternalInput").ap()
    bqk_ap = nc.dram_tensor("b_qk_col", [128, MT], F32, kind="ExternalInput").ap()
    bv_ap = nc.dram_tensor("b_v_row", [1, D], BF16, kind="ExternalInput").ap()
    bp_ap = nc.dram_tensor("b_p_row", [1, D], BF16, kind="ExternalInput").ap()
    ones_bf_ap = nc.dram_tensor("ones_bf", [1, 128], BF16, kind="ExternalInput").ap()
    out_ap = nc.dram_tensor("out", [T, D], F32, kind="ExternalOutput").ap()

    with tile.TileContext(nc) as tc, contextlib.ExitStack() as ctx:
        p_xT = ctx.enter_context(tc.tile_pool(name="xT", bufs=2 * KT))
        p_w = ctx.enter_context(tc.tile_pool(name="w", bufs=KT))
        p_wqk = ctx.enter_context(tc.tile_pool(name="wqk", bufs=3))
        p_vsb = ctx.enter_context(tc.tile_pool(name="vsb", bufs=2 * TT))
        p_qk = ctx.enter_context(tc.tile_pool(name="qk", bufs=2 * H))
        p_qksb = ctx.enter_context(tc.tile_pool(name="qksb", bufs=2))
        p_expT = ctx.enter_context(tc.tile_pool(name="expT", bufs=8))
        p_pvs = ctx.enter_context(tc.tile_pool(name="pvs", bufs=2))
        p_at = ctx.enter_context(tc.tile_pool(name="at", bufs=2))
        p_apk = ctx.enter_context(tc.tile_pool(name="apk", bufs=2 * KT))
        p_ost = ctx.enter_context(tc.tile_pool(name="ost", bufs=3))
        p_rec = ctx.enter_context(tc.tile_pool(name="rec", bufs=2))
        p_bias = ctx.enter_context(tc.tile_pool(name="bias", bufs=1))
        p_brow = ctx.enter_context(tc.tile_pool(name="brow", bufs=2))
        p_const = ctx.enter_context(tc.tile_pool(name="const", bufs=1))

        # PSUM: 8 banks = 2x2 (sc: scores 2-deep + pb) + 1x2 (pv) + 1x2 (big)
        p_sc = ctx.enter_context(tc.tile_pool(name="psc", bufs=2, space="PSUM"))
        p_pv = ctx.enter_context(tc.tile_pool(name="ppv", bufs=1, space="PSUM"))
        p_big = ctx.enter_context(tc.tile_pool(name="pbig", bufs=1, space="PSUM"))

        # ---- x^T tiles for b0 FIRST: stage B's critical path ----
        xTb = {0: [None] * KT, 1: [None] * KT}
        for k in range(KT):
            t = p_xT.tile([128, SP], BF16, tag="xT", name=f"xT_0_{k}")
            eng = nc.sync if k % 2 == 0 else nc.scalar
            eng.dma_start(t[:], xT_ap[k * 128:(k + 1) * 128, 0:SP])
            xTb[0][k] = t

        # ---- constants (not needed until ~12us in) ----
        ones_bf = p_const.tile([1, 128], BF16, tag="ones_bf")
        nc.sync.dma_start(ones_bf[:], ones_bf_ap[0:1, :])
        bvr = p_brow.tile([1, D], BF16, tag="brow", name="bvr")
        nc.sync.dma_start(bvr[:], bv_ap[:])
        bpr = p_brow.tile([1, D], BF16, tag="brow", name="bpr")
        nc.sync.dma_start(bpr[:], bp_ap[:])
        bq_all = p_bias.tile([128, MT], F32, tag="bias", name="bq_all")
        nc.sync.dma_start(bq_all[:], bqk_ap[:])

        # ---- x^T tiles for b1 (gpsimd queue: idle once wv lands, and
        # keeps these off the sync lane that stage B(b1) was stalling on) ----
        for k in range(KT):
            t = p_xT.tile([128, SP], BF16, tag="xT", name=f"xT_1_{k}")
            nc.gpsimd.dma_start(t[:], xT_ap[k * 128:(k + 1) * 128, SP:2 * SP])
            xTb[1][k] = t

        # ---- resident v-proj weights (11 tiles; wp reuses the slots) ----
        wv = []
        for k in range(KT):
            t = p_w.tile([128, D], BF16, tag="w", name=f"wv{k}")
            nc.gpsimd.dma_start(t[:], wv_ap[:, k * D:(k + 1) * D])
            wv.append(t)

        # ---- v tiles: ones-filled via gpsimd memset (denominator cols) ----
        vsb = {}
        for b in range(B):
            vsb[b] = []
            for tt in range(TT):
                t = p_vsb.tile([128, H * VG], BF16, tag="vsb",
                               name=f"vsb_{b}_{tt}")
                nc.vector.memset(t[:], 1.0)
                vsb[b].append(t)

        qkt = {}

        def qk_tile(b, which, h):
            key = (b, which, h)
            if key not in qkt:
                qkt[key] = p_qk.tile([HD, SP], BF16, tag="qk",
                                     name=f"qk_{b}_{which}_{h}")
            return qkt[key]

        apk = {b: [p_apk.tile([128, SP], BF16, tag="apk", name=f"apk_{b}_{k}")
                   for k in range(KT)] for b in range(B)}

        # ================= piece generators =================

        def gen_b_chunk(b, ci):
            """v projection for batch b, feature chunk ci (token-major)."""
            c0, w = CH_D[ci]
            boff = b * SP
            yield 0
            sA = p_sc.tile([128, 1024], F32, tag="ps", name=f"bps_{b}_{ci}_a")
            sB = p_sc.tile([128, 1024], F32, tag="ps", name=f"bps_{b}_{ci}_b")
            sC = p_pv.tile([128, 1024], F32, tag="ps", name=f"bps_{b}_{ci}_c")
            slot = [(sA, 0), (sA, 512), (sB, 0), (sB, 512), (sC, 0)]
            for k in range(KT):
                for tt, t0, ts in TOK:
                    tl, off = slot[tt]
                    nc.tensor.matmul(tl[0:ts, off:off + w],
                                     xTb[b][k][:, t0:t0 + ts],
                                     wv[k][:, c0:c0 + w],
                                     start=(k == 0), stop=False)
                yield TT * w
            for tt, t0, ts in TOK:
                tl, off = slot[tt]
                nc.tensor.matmul(tl[0:ts, off:off + w], ones_bf[:, 0:ts],
                                 bvr[:, c0:c0 + w], start=False, stop=True)
            yield TT * w
            h0, h1 = c0 // HD, min(H - 1, (c0 + w - 1) // HD)
            for tt, t0, ts in TOK:
                tl, off = slot[tt]
                for h in range(h0, h1 + 1):
                    s0, s1 = max(c0, h * HD), min(c0 + w, (h + 1) * HD)
                    if s1 <= s0:
                        continue
                    nc.vector.tensor_copy(
                        vsb[b][tt][0:ts, h * VG + (s0 - h * HD):
                                   h * VG + (s1 - h * HD)],
                        tl[0:ts, off + (s0 - c0):off + (s1 - c0)])
                yield 0

        def gen_c1_m(b, m):
            """q|k projection m-tile (feature-major) + head redistribution."""
            wq = p_wqk.tile([128, KT * 128], BF16, tag="wqk", name=f"wq_{b}_{m}")
            nc.scalar.dma_start(wq[:], wqkm_ap[:, m * 1408:(m + 1) * 1408])
            yield 0
            boff = b * SP
            pt = p_big.tile([128, 1024], F32, tag="ps", name=f"c1p_{b}_{m}")
            for k in range(KT):
                nc.tensor.matmul(pt[0:128, 0:512], wq[:, k * 128:(k + 1) * 128],
                                 xTb[b][k][:, 0:512],
                                 start=(k == 0), stop=(k == KT - 1))
                nc.tensor.matmul(pt[0:128, 512:578], wq[:, k * 128:(k + 1) * 128],
                                 xTb[b][k][:, 512:578],
                                 start=(k == 0), stop=(k == KT - 1))
                yield SP
            qksb = p_qksb.tile([128, SP], BF16, tag="qksb", name=f"qs_{b}_{m}")
            nc.vector.tensor_scalar_add(qksb[:, 0:SP], pt[0:128, 0:SP],
                                        bq_all[:, m:m + 1])
            which = 0 if m < KT else 1
            f_lo = (m - which * KT) * 128
            f_hi = f_lo + 128
            for h in range(f_lo // HD, min(H, (f_hi + HD - 1) // HD)):
                s0, s1 = max(f_lo, h * HD), min(f_hi, (h + 1) * HD)
                if s1 <= s0:
                    continue
                t = qk_tile(b, which, h)
                r0 = s0 - h * HD
                nc.sync.dma_start(t[r0:r0 + (s1 - s0), :],
                                  qksb[s0 - f_lo:s1 - f_lo, :])
            yield 0

        wp = []

        def emit_wp_load():
            k = len(wp)
            if k < KT:
                t = p_w.tile([128, D], BF16, tag="w", name=f"wp{k}")
                nc.gpsimd.dma_start(t[:], wp_ap[:, k * D:(k + 1) * D])
                wp.append(t)

        def gen_d_piece(b, tt, t0, ts, cgi, pool):
            """out-proj for batch b, token tile tt, column group cgi."""
            c0, w = DCG[cgi]
            yield 0
            po = pool.tile([128, 1024], F32, tag="ps", name=f"po_{b}_{tt}_{cgi}")
            for k in range(KT):
                if w == 1024:
                    nc.tensor.matmul(po[0:ts, 0:512], apk[b][k][:, t0:t0 + ts],
                                     wp[k][:, c0:c0 + 512],
                                     start=(k == 0), stop=False)
                    nc.tensor.matmul(po[0:ts, 512:1024], apk[b][k][:, t0:t0 + ts],
                                     wp[k][:, c0 + 512:c0 + 1024],
                                     start=(k == 0), stop=False)
                else:
                    nc.tensor.matmul(po[0:ts, 0:w], apk[b][k][:, t0:t0 + ts],
                                     wp[k][:, c0:c0 + w],
                                     start=(k == 0), stop=False)
                yield w
            if w == 1024:
                nc.tensor.matmul(po[0:ts, 0:512], ones_bf[:, 0:ts],
                                 bpr[:, c0:c0 + 512], start=False, stop=True)
                nc.tensor.matmul(po[0:ts, 512:1024], ones_bf[:, 0:ts],
                                 bpr[:, c0 + 512:c0 + 1024], start=False, stop=True)
            else:
                nc.tensor.matmul(po[0:ts, 0:w], ones_bf[:, 0:ts],
                                 bpr[:, c0:c0 + w], start=False, stop=True)
            ot = p_ost.tile([128, 1024], F32, tag="ost", name=f"ot_{b}_{tt}_{cgi}")
            nc.vector.tensor_copy(ot[0:ts, 0:w], po[0:ts, 0:w])
            nc.sync.dma_start(out_ap[b * S + t0:b * S + t0 + ts, c0:c0 + w],
                              ot[0:ts, 0:w])
            yield 0

        # ================= attention =================

        def finish_head(b, h, rec, pvs):
            pb = p_sc.tile([128, 1024], F32, tag="ps", name=f"pb_{b}_{h}")
            nc.tensor.matmul(pb[0:HD, 0:512], ones_bf[:, 0:HD],
                             rec[:, 0:512], start=True, stop=True)
            nc.tensor.matmul(pb[0:HD, 512:578], ones_bf[:, 0:HD],
                             rec[:, 512:578], start=True, stop=True)
            at = p_at.tile([HD, SP], BF16, tag="at", name=f"at_{b}_{h}")
            nc.vector.tensor_mul(at[:, 0:SP], pvs[0:HD, 0:SP], pb[0:HD, 0:SP])
            f0 = h * HD
            k0, r0 = f0 // 128, f0 % 128
            n0 = min(HD, 128 - r0)
            nc.sync.dma_start(apk[b][k0][r0:r0 + n0, :], at[0:n0, :])
            if n0 < HD:
                nc.sync.dma_start(apk[b][k0 + 1][0:HD - n0, :], at[n0:HD, :])

        def attention_phase(b, filler, fill_exp, fill_pv, gate=None):
            pend = None
            for h in range(H):
                if gate is not None:
                    gate(h)
                qh_t = qk_tile(b, 0, h)
                kh_t = qk_tile(b, 1, h)
                expTs = []
                for tt, t0, ts in TOK:
                    pt = p_sc.tile([128, 1024], F32, tag="ps",
                                   name=f"sc_{b}_{h}_{tt}")
                    nc.tensor.matmul(pt[0:ts, 0:512], kh_t[:, t0:t0 + ts],
                                     qh_t[:, 0:512], start=True, stop=True)
                    nc.tensor.matmul(pt[0:ts, 512:578], kh_t[:, t0:t0 + ts],
                                     qh_t[:, 512:578], start=True, stop=True)
                    et = p_expT.tile([128, SP], BF16, tag="expT",
                                     name=f"et_{b}_{h}_{tt}")
                    nc.scalar.activation(et[0:ts, 0:SP], pt[0:ts, 0:SP],
                                         AF.Exp, scale=SCALE)
                    expTs.append(et)
                    filler.take(fill_exp)
                pv = p_pv.tile([128, 1024], F32, tag="ps", name=f"pv_{b}_{h}")
                for tt, t0, ts in TOK:
                    et = expTs[tt]
                    vsl = vsb[b][tt][0:ts, h * VG:(h + 1) * VG]
                    nc.tensor.matmul(pv[0:VG, 0:512], vsl, et[0:ts, 0:512],
                                     start=(tt == 0), stop=(tt == TT - 1))
                    nc.tensor.matmul(pv[0:VG, 512:578], vsl, et[0:ts, 512:578],
                                     start=(tt == 0), stop=(tt == TT - 1))
                    filler.take(fill_pv)
                # one copy brings values AND the denominator row to SBUF
                pvs = p_pvs.tile([VG, SP], F32, tag="pvs", name=f"pvs_{b}_{h}")
                nc.vector.tensor_copy(pvs[:, 0:SP], pv[0:VG, 0:SP])
                # custom-DVE op needs base partition 0: run it over all VG
                # rows (same per-lane cost); only row 96 (the denominator)
                # is meaningful.
                recf = p_rec.tile([VG, SP], F32, tag="recf", name=f"recf_{b}_{h}")
                nc.vector.reciprocal_approx_fast(recf[:, 0:SP], pvs[:, 0:SP])
                rec = p_rec.tile([1, SP], BF16, tag="rec", name=f"rec_{b}_{h}")
                nc.vector.tensor_copy(rec[:, 0:SP], recf[DEN:DEN + 1, 0:SP])
                if pend is not None:
                    finish_head(b, *pend)
                pend = (h, rec, pvs)
            finish_head(b, *pend)

        # ================= phase drivers =================

        def drain(gen):
            for _ in gen:
                pass

        # ---- P1: v-proj(b0,b1) zipped with q|k-proj(b0) at ~1.2us grain ----
        # Fine interleave keeps the PE fed while the single big-psum slot
        # drains between C1 m-tiles (and vice versa for B's psum slots).
        fB = Filler()
        for b in range(B):
            for ci in range(3):
                fB.add(gen_b_chunk(b, ci))
        fC = Filler()
        for m in range(MT):
            fC.add(gen_c1_m(0, m))
        fB.take(31000)          # all of B(b0, chunk0) before any C1
        while fB.q or fC.q:
            fB.take(3100)
            fC.take(2560)

        # ---- P2: attention(b0) with q|k-proj(b1) as filler ----
        f2 = Filler()
        c1b1 = [gen_c1_m(1, m) for m in range(MT)]
        gstate = {"j": 0}

        for _ in range(KT):
            emit_wp_load()

        def gate2(h):
            jm = min((88 * (h + 2)) // 128, KT - 1)
            while gstate["j"] <= jm:
                j = gstate["j"]
                f2.add(c1b1[j])          # q m-tile j
                f2.add(c1b1[KT + j])     # k m-tile j
                gstate["j"] += 1

        attention_phase(0, f2, fill_exp=1100, fill_pv=650, gate=gate2)
        f2.drain()

        # ---- P3: attention(b1) with out-proj(b0) as filler ----
        f3 = Filler()
        for tt, t0, ts in TOK:
            for cgi in range(2):
                f3.add(gen_d_piece(0, tt, t0, ts, cgi, p_big))
        attention_phase(1, f3, fill_exp=650, fill_pv=390)
        f3.drain()

        # ---- P4: out-proj(b1), psum rotating across all three pools ----
        pools4 = [p_big, p_sc, p_pv]
        pi = 0
        d4 = []
        for tt, t0, ts in TOK:
            for cgi in range(2):
                d4.append(gen_d_piece(1, tt, t0, ts, cgi, pools4[pi % 3]))
                pi += 1
        primed = 0
        for i in range(len(d4)):
            while primed < min(i + 2, len(d4)):
                try:
                    next(d4[primed])
                except StopIteration:
                    pass
                primed += 1
            drain(d4[i])

    nc.compile()
    return nc


_NC_CACHE = None


def _get_nc():
    global _NC_CACHE
    if _NC_CACHE is None:
        _NC_CACHE = build_program()
    return _NC_CACHE


def make_in_maps(hidden_states, w_qkv, b_qkv, w_proj, b_proj):
    bf16 = ml_dtypes.bfloat16
    hs = np.asarray(hidden_states, dtype=np.float32)
    w_qkv = np.ascontiguousarray(np.asarray(w_qkv, dtype=np.float32))
    b_qkv = np.asarray(b_qkv, dtype=np.float32)
    w_proj = np.ascontiguousarray(np.asarray(w_proj, dtype=np.float32))
    b_proj = np.asarray(b_proj, dtype=np.float32)

    # q|k weights -> m-stripe layout: wqk_m[p, m*1408 + k*128 + c]
    #   = w_qkv[k*128 + p, m*128 + c]
    wqk = w_qkv[:, :2 * D].reshape(KT, 128, MT, 128)
    wqk_m = np.ascontiguousarray(
        wqk.transpose(1, 2, 0, 3).reshape(128, MT * KT * 128)).astype(bf16)
    # v / proj weights -> k-stripe layout: w_r[p, k*1408 + c] = w[k*128+p, c]
    wv_r = np.ascontiguousarray(
        w_qkv[:, 2 * D:].reshape(KT, 128, D).transpose(1, 0, 2)
        .reshape(128, KT * D)).astype(bf16)
    wp_r = np.ascontiguousarray(
        w_proj.reshape(KT, 128, D).transpose(1, 0, 2)
        .reshape(128, KT * D)).astype(bf16)

    bqk_m = np.ascontiguousarray(
        b_qkv[:2 * D].reshape(MT, 128).T)                 # [128, MT]
    bv_row = np.ascontiguousarray(b_qkv[2 * D:].reshape(1, D)).astype(bf16)
    bp_row = np.ascontiguousarray(b_proj.reshape(1, D)).astype(bf16)
    ones_bf = np.ones((1, 128), bf16)

    in_maps = []
    for c in range(N_CORES):
        xt = np.ones((D, B * SP), bf16)
        for b in range(B):
            xs = hs[c * B + b]                       # [S, D]
            xt[:, b * SP:b * SP + S] = xs.T.astype(bf16)
        in_maps.append({
            "xT_bf": xt,
            "wqk_m": wqk_m,
            "wv_r": wv_r,
            "wp_r": wp_r,
            "b_qk_col": bqk_m,
            "b_v_row": bv_row,
            "b_p_row": bp_row,
            "ones_bf": ones_bf,
        })
    return in_maps


def kernel(hidden_states, w_qkv, b_qkv, w_proj, b_proj):
    nc = _get_nc()
    in_maps = make_in_maps(hidden_states, w_qkv, b_qkv, w_proj, b_proj)
    res = run_bass_kernel_spmd(nc, in_maps, list(range(N_CORES)))
    out = np.concatenate(
        [res.results[c]["out"].reshape(B, S, D) for c in range(N_CORES)], axis=0)
    return out.astype(np.float32)


if __name__ == "__main__":
    rng = np.random.default_rng(0)
    hs = rng.standard_normal((B_TOTAL, S, D), dtype=np.float32)
    wq = rng.standard_normal((D, 3 * D), dtype=np.float32) * D ** -0.5
    bq = rng.standard_normal(3 * D).astype(np.float32) * 0.02
    wp = rng.standard_normal((D, D), dtype=np.float32) * D ** -0.5
    bp = rng.standard_normal(D).astype(np.float32) * 0.02
    o = kernel(hidden_states=hs, w_qkv=wq, b_qkv=bq, w_proj=wp, b_proj=bp)
    print(o.shape, o.dtype)
